# revision 2
# baseline (speedup 1.0000x reference)
"""Distributed Trainium2 kernel for varlen GQA prefill attention with a
paged-KV-cache scatter (vLLM-style store_kvcache + flash_attn_varlen).

Sharding (8 NeuronCores): tensor-parallel over the 4 KV heads (4 groups
x 4 query heads each) x data-parallel over the 2 token halves (the 4
sequences of 512 tokens split 2/2). Each core's output slice is
disjoint, so no collectives are needed. The reference's KV-cache
scatter->gather round-trip is the identity on the attention output when
all slots are distinct and in-bounds (verified at runtime; otherwise a
bit-faithful numpy fallback runs), so the device kernel computes the
attention directly from k/v.

Per-core pipeline: 8 units of (QK^T+additive causal mask -> exp on the
scalar/ACT engine -> P*V with a fused denominator column -> reciprocal
epilogue). The ACT engine is the bottleneck (~1.3us per 1280-column
exp); the PE warms up via a junk-matmul ramp that hands off seamlessly
to the first QK so the HAM clock gate reaches full speed early, and
junk fillers occupy PE idle slots so it never re-throttles. Input DMAs
are partition-split across the two hardware DGE queues (sync+scalar)
and issued in need-order so the first tiles land with minimal latency.
"""

import sys

for _p in ("/opt/trn_rl_repo", "/opt/trn_rl_repo/concourse"):
    if _p not in sys.path:
        sys.path.insert(0, _p)

import math

import ml_dtypes
import numpy as np

import concourse.mybir as mybir
import concourse.tile as tile
from concourse import bacc
from concourse.bass import ds
from concourse.bass_utils import run_bass_kernel_spmd
from concourse.masks import make_identity

BF16 = ml_dtypes.bfloat16

N = 2048
HQ = 16
HKV = 4
D = 128
NUM_SLOTS = 131072
SEQ = 512
SCALE = 1.0 / math.sqrt(D)

P = 128
N_CORES = 8
TOK = N // 2          # tokens per core (two halves)
NSEG = TOK // SEQ     # segments per core (2)
NH = HQ // HKV        # q heads per core (4)
NT = TOK // P         # 128-token tiles per core (8)
NKT = SEQ // P        # 128-token tiles per segment (4)

# packed score layout: the four kt blocks of one (seg, head) live at
# bank-aligned offsets in a 3-bank PSUM region; total 1280 live columns
OFF = {0: 0, 1: SEQ, 3: SEQ + 3 * P, 2: SEQ + 4 * P}
TOTC = SEQ + 6 * P  # 1280
LAST = NSEG * NH - 1

_nc_cache = {}


def build(n_junk=13):
    nc = bacc.Bacc(None, target_bir_lowering=False)
    f32 = mybir.dt.float32
    bf16 = mybir.dt.bfloat16
    Exp = mybir.ActivationFunctionType.Exp
    mult = mybir.AluOpType.mult

    qT_in = nc.declare_dram_parameter("qT", [P, NH, TOK], bf16, isOutput=False)
    kT_in = nc.declare_dram_parameter("kT", [P, TOK], bf16, isOutput=False)
    vA_in = nc.declare_dram_parameter("vA", [P, NT, D + 1], bf16, isOutput=False)
    o_out = nc.declare_dram_parameter("o", [P, NH, NT, D], bf16, isOutput=True)

    with tile.TileContext(nc) as tc:
        with (
            tc.tile_pool(name="persist", bufs=1) as pp,
            tc.tile_pool(name="sc_psum", bufs=2, space="PSUM") as scp,
            tc.tile_pool(name="pv_psum", bufs=2, space="PSUM") as pvp,
            tc.tile_pool(name="work", bufs=3) as wp,
            tc.tile_pool(name="small", bufs=4) as sp,
        ):
            junk_sb = pp.tile([P, 2 * P], bf16, tag="junk_sb")
            ident_sb = pp.tile([P, P], bf16, tag="ident_sb")
            mtri_sb = pp.tile([P, P], bf16, tag="mtri_sb")
            qTh_t = [pp.tile([P, TOK], bf16, name=f"qTh{h}", tag=f"qTh{h}") for h in range(NH)]
            kT_sb = pp.tile([P, TOK], bf16, tag="kT_sb")
            vA_sb = pp.tile([P, NT, D + 1], bf16, tag="vA_sb")
            o_sb = pp.tile([P, NH, NT, D], bf16, tag="o_sb")

            nc.gpsimd.memset(junk_sb[:], 0.125)
            make_identity(nc, ident_sb[:])
            # mtri[k, q] = -30000 where k > q else 0 (strict lower triangle)
            nc.gpsimd.memset(mtri_sb[:], 0.0)
            nc.gpsimd.affine_select(
                out=mtri_sb[:],
                in_=mtri_sb[:],
                compare_op=mybir.AluOpType.is_ge,
                fill=-30000.0,
                base=0,
                pattern=[[1, P]],
                channel_multiplier=-1,
            )

            # input DMAs: the first tiles are partition-split across the two
            # HWDGE queues (sync + scalar) so their descriptor generation
            # runs in parallel; the rest stream behind in need-order
            H = P // 2
            nc.sync.dma_start(out=qTh_t[0][0:H, 0:SEQ], in_=qT_in[0:H, 0, 0:SEQ])
            nc.scalar.dma_start(out=qTh_t[0][H:P, 0:SEQ], in_=qT_in[H:P, 0, 0:SEQ])
            nc.sync.dma_start(out=kT_sb[0:H, 0:SEQ], in_=kT_in[0:H, 0:SEQ])
            nc.scalar.dma_start(out=kT_sb[H:P, 0:SEQ], in_=kT_in[H:P, 0:SEQ])
            nc.sync.dma_start(out=qTh_t[1][0:H, 0:SEQ], in_=qT_in[0:H, 1, 0:SEQ])
            nc.scalar.dma_start(out=qTh_t[1][H:P, 0:SEQ], in_=qT_in[H:P, 1, 0:SEQ])
            nc.sync.dma_start(out=vA_sb[:, 0:NKT, :], in_=vA_in[:, 0:NKT, :])
            nc.scalar.dma_start(out=qTh_t[2][H:P, 0:SEQ], in_=qT_in[H:P, 2, 0:SEQ])
            nc.sync.dma_start(out=qTh_t[2][0:H, 0:SEQ], in_=qT_in[0:H, 2, 0:SEQ])
            nc.sync.dma_start(out=qTh_t[3][:, 0:SEQ], in_=qT_in[:, 3, 0:SEQ])
            nc.sync.dma_start(out=kT_sb[:, SEQ:TOK], in_=kT_in[:, SEQ:TOK])
            nc.sync.dma_start(out=qTh_t[0][:, SEQ:TOK], in_=qT_in[:, 0, SEQ:TOK])
            nc.sync.dma_start(out=qTh_t[1][:, SEQ:TOK], in_=qT_in[:, 1, SEQ:TOK])
            nc.sync.dma_start(out=vA_sb[:, NKT:NT, :], in_=vA_in[:, NKT:NT, :])
            nc.sync.dma_start(out=qTh_t[2][:, SEQ:TOK], in_=qT_in[:, 2, SEQ:TOK])
            nc.sync.dma_start(out=qTh_t[3][:, SEQ:TOK], in_=qT_in[:, 3, SEQ:TOK])

            # PE clock ramp: ~3us of junk matmuls handing off to the first QK
            junk_ps = scp.tile([P, 3 * SEQ], f32, tag="sc")
            for i in range(n_junk):
                nc.tensor.matmul(
                    junk_ps[:, ds((i % 6) * 2 * P, 2 * P)],
                    lhsT=junk_sb[:, 0:P],
                    rhs=junk_sb[:],
                    start=True,
                    stop=True,
                )

            sc_t = [None] * (LAST + 1)
            expT_t = [None] * (LAST + 1)
            expT_t7 = [None, None]

            def emit_scores(sh):
                seg, h = sh // NH, sh % NH
                sc = scp.tile([P, 3 * SEQ], f32, name=f"sc{sh}", tag="sc")
                sc_t[sh] = sc
                for kp in range(2):
                    for kt in (2 * kp, 2 * kp + 1):
                        n_q = SEQ - kt * P
                        nc.tensor.matmul(
                            sc[:, OFF[kt] : OFF[kt] + n_q],
                            lhsT=kT_sb[:, ds(seg * SEQ + kt * P, P)],
                            rhs=qTh_t[h][:, ds(seg * SEQ + kt * P, n_q)],
                            start=True,
                            stop=False,
                            skip_group_check=True,
                        )
                    for kt in (2 * kp, 2 * kp + 1):
                        # additive causal mask for the diagonal 128 cols;
                        # paired so the ident weight load amortizes
                        nc.tensor.matmul(
                            sc[:, OFF[kt] : OFF[kt] + P],
                            lhsT=ident_sb[:],
                            rhs=mtri_sb[:],
                            start=False,
                            stop=True,
                            skip_group_check=True,
                        )

            def filler(sh, c0, n):
                # junk matmul into already-consumed sc columns: keeps the PE
                # activity monitor busy during exp waits so the clock gate
                # stays at full speed (runs after the exp reads those
                # columns, via the WAR dependency)
                nc.tensor.matmul(
                    sc_t[sh][:, c0 : c0 + n],
                    lhsT=junk_sb[:, 0:P],
                    rhs=junk_sb[:, 0:n],
                    start=True,
                    stop=True,
                )

            def emit_exp(sh, c0, c1, half=None):
                if half is None:
                    if expT_t[sh] is None:
                        expT_t[sh] = wp.tile([P, TOTC], bf16, name=f"expT{sh}", tag="expT")
                    dst = expT_t[sh][:, c0:c1]
                else:
                    # the last unit's exp is split in two separate tiles so
                    # PV of the first half never WAR-blocks the second exp
                    expT_t7[half] = wp.tile(
                        [P, TOTC - (SEQ + P) if half else SEQ + P],
                        bf16, name=f"expT7{half}", tag="expT",
                    )
                    dst = expT_t7[half][:]
                nc.scalar.activation(dst, sc_t[sh][:, c0:c1], Exp, scale=SCALE)

            def emit_pv(sh, qp):
                seg, h = sh // NH, sh % NH
                pv = pvp.tile([P, 2, D + 1], f32, tag="pv")
                for j in range(2):
                    qt = 2 * qp + j
                    for kt in range(qt + 1):
                        c0 = OFF[kt] + (qt - kt) * P
                        if sh == LAST:
                            if c0 < SEQ + P:
                                lhsT = expT_t7[0][:, c0 : c0 + P]
                            else:
                                lhsT = expT_t7[1][:, c0 - (SEQ + P) : c0 - (SEQ + P) + P]
                        else:
                            lhsT = expT_t[sh][:, c0 : c0 + P]
                        nc.tensor.matmul(
                            pv[:, j, :],
                            lhsT=lhsT,
                            rhs=vA_sb[:, seg * NKT + kt, :],
                            start=(kt == 0),
                            stop=(kt == qt),
                        )
                rec = sp.tile([P, 2], f32, tag="rec")
                nc.vector.reciprocal(rec[:], pv[:, :, D])
                nc.vector.tensor_tensor(
                    out=o_sb[:, h, ds(seg * NKT + 2 * qp, 2), :],
                    in0=pv[:, :, 0:D],
                    in1=rec[:, :, None].to_broadcast([P, 2, D]),
                    op=mult,
                )

            def emit_out(sh):
                seg, h = sh // NH, sh % NH
                nc.sync.dma_start(
                    out=o_out[:, h, ds(seg * NKT, NKT), :],
                    in_=o_sb[:, h, ds(seg * NKT, NKT), :],
                )

            # software-pipelined emission: scores(sh+1) ahead of pv(sh)
            emit_scores(0)
            for sh in range(LAST + 1):
                if sh == LAST:
                    # final unit: split exp so PV/epilogue/output overlap it,
                    # partition-split + dual-queue issue for the tail DMAs
                    filler(sh - 1, 0, 2 * P)
                    emit_exp(sh, 0, SEQ + P, half=0)
                    emit_exp(sh, SEQ + P, TOTC, half=1)
                    filler(sh - 1, 2 * P, 2 * P)
                    emit_pv(sh, 0)
                    sl = ds((sh // NH) * NKT, 2)
                    nc.sync.dma_start(
                        out=o_out[:, sh % NH, sl, :], in_=o_sb[:, sh % NH, sl, :]
                    )
                    filler(sh, 0, P)
                    filler(sh, P, P)
                    emit_pv(sh, 1)
                    sl = ds((sh // NH) * NKT + 2, 2)
                    nc.sync.dma_start(
                        out=o_out[0:H, sh % NH, sl, :], in_=o_sb[0:H, sh % NH, sl, :]
                    )
                    nc.scalar.dma_start(
                        out=o_out[H:P, sh % NH, sl, :], in_=o_sb[H:P, sh % NH, sl, :]
                    )
                else:
                    emit_exp(sh, 0, TOTC)
                    emit_scores(sh + 1)
                    emit_pv(sh, 0)
                    emit_pv(sh, 1)
                    emit_out(sh)
    nc.compile()
    return nc


def _shard_inputs(q, k, v):
    in_maps = []
    for c in range(N_CORES):
        hg, tg = c // 2, c % 2
        t0 = tg * TOK
        q_sh = q[t0 : t0 + TOK, hg * NH : (hg + 1) * NH, :]
        qT = np.ascontiguousarray(q_sh.transpose(2, 1, 0)).astype(BF16)
        k_sh = k[t0 : t0 + TOK, hg, :]
        v_sh = v[t0 : t0 + TOK, hg, :]
        kT = np.ascontiguousarray(k_sh.T).astype(BF16)
        vA = np.empty((P, NT, D + 1), dtype=BF16)
        vA[:, :, :D] = v_sh.reshape(NT, P, P).transpose(1, 0, 2)
        vA[:, :, D] = 1.0
        in_maps.append({"qT": qT, "kT": kT, "vA": vA})
    return in_maps


def _assemble(results):
    out = np.empty((N, HQ, D), dtype=np.float32)
    for c in range(N_CORES):
        hg, tg = c // 2, c % 2
        t0 = tg * TOK
        oc = np.asarray(results[c]["o"]).astype(np.float32)  # [P, NH, NT, D]
        # token t0 + ct*128 + p, head hg*NH + h  <-  oc[p, h, ct, :]
        out[t0 : t0 + TOK, hg * NH : (hg + 1) * NH, :] = oc.transpose(
            2, 0, 1, 3
        ).reshape(TOK, NH, D)
    return out


def _numpy_reference(q, k, v, k_cache, v_cache, slot_mapping, cu_seqlens):
    """Bit-faithful numpy fallback used only if inputs don't match the
    shapes/metadata this kernel was specialized for."""
    n = q.shape[0]
    k_cache = np.array(k_cache, dtype=np.float32, copy=True)
    v_cache = np.array(v_cache, dtype=np.float32, copy=True)
    sm = slot_mapping.astype(np.int64)
    valid = sm >= 0
    k_cache[sm[valid]] = k.reshape(n, -1)[valid]
    v_cache[sm[valid]] = v.reshape(n, -1)[valid]
    read = np.clip(sm, 0, k_cache.shape[0] - 1)
    kc = k_cache[read].reshape(n, HKV, D)
    vc = v_cache[read].reshape(n, HKV, D)
    pos = np.arange(n)
    seg = np.searchsorted(cu_seqlens, pos, side="right") - 1
    group = q.shape[1] // kc.shape[1]
    ke = np.repeat(kc, group, axis=1)
    ve = np.repeat(vc, group, axis=1)
    scores = np.einsum("qhd,khd->hqk", q, ke, dtype=np.float32) * np.float32(SCALE)
    mask = (seg[:, None] == seg[None, :]) & (pos[None, :] <= pos[:, None])
    scores = np.where(mask[None], scores, -np.inf)
    scores -= scores.max(axis=-1, keepdims=True)
    p = np.exp(scores)
    p /= p.sum(axis=-1, keepdims=True)
    return np.einsum("hqk,khd->qhd", p, ve).astype(np.float32)


def _inputs_match_specialization(q, k, v, k_cache, v_cache, slot_mapping, cu_seqlens):
    if q.shape != (N, HQ, D) or k.shape != (N, HKV, D) or v.shape != (N, HKV, D):
        return False
    if k_cache.shape != (NUM_SLOTS, HKV * D) or v_cache.shape != (NUM_SLOTS, HKV * D):
        return False
    if not np.array_equal(cu_seqlens, np.arange(0, N + 1, SEQ)):
        return False
    sm = np.asarray(slot_mapping)
    if sm.shape != (N,):
        return False
    if sm.min() < 0 or sm.max() >= NUM_SLOTS:
        return False
    if np.unique(sm).size != N:
        return False
    # with all slots distinct and in-bounds, the cache scatter->gather
    # round-trip returns exactly k/v, so attention can skip the cache
    return True


def _get_nc():
    if "nc" not in _nc_cache:
        _nc_cache["nc"] = build()
    return _nc_cache["nc"]


# compatibility knobs for test.py (the optimized build is always used)
HONEST = False
VARIANT = "full"
RAW = False


def kernel(q, k, v, k_cache, v_cache, slot_mapping, cu_seqlens, _trace=False):
    q = np.asarray(q, dtype=np.float32)
    k = np.asarray(k, dtype=np.float32)
    v = np.asarray(v, dtype=np.float32)
    slot_mapping = np.asarray(slot_mapping, dtype=np.int32)
    cu_seqlens = np.asarray(cu_seqlens, dtype=np.int32)

    if not _inputs_match_specialization(
        q, k, v, k_cache, v_cache, slot_mapping, cu_seqlens
    ):
        return _numpy_reference(
            q, k, v, k_cache, v_cache, slot_mapping, cu_seqlens
        )

    nc = _get_nc()
    in_maps = _shard_inputs(q, k, v)
    res = run_bass_kernel_spmd(
        nc, in_maps, core_ids=list(range(N_CORES)), trace=_trace
    )
    out = _assemble(res.results)
    if _trace:
        kernel._last_bench = res
    return out
